# revision 4
# baseline (speedup 1.0000x reference)
"""Depthwise causal Conv1d (k=4) + SiLU on 8 Trainium2 NeuronCores.

Problem: x [4, 4096, 2048] f32, w [2048, 4] f32,
out[b, t, d] = silu(sum_j w[d, j] * x[b, t - 3 + j, d])   (zero-padded left).

Sharding: 8 cores = 4 batches x 2 channel-halves. Depthwise conv is
independent per channel, so channel sharding needs no halo exchange.

Layout: each core receives its shard host-transposed to [channels, time]
(channels on SBUF partitions). The per-channel weight w[d, j] is a
per-partition scalar and the causal time shifts are free-dim AP offsets
into one loaded tile.

The kernel is HBM-bandwidth-bound: ~16.4 MB of fp16 I/O per core, and the
16 SDMA engines move ~1 MB each at their ~27 GiB/s line rate, so the floor
is startup (~7.5 us) + ~43 us of streaming + store-receipt/fence tail.
Schedule:
 - All loads queue back to back at the head of the sync HWDGE ring
   (pure reads, no stalls); per-block stores queue on the SAME ring
   afterwards, each gated by a tiny GpSimd bypass op that reads the
   LAST x tile, so store descriptor-gen starts right as the load stream
   drains (not earlier - that would steal load bandwidth and starve
   compute; not later - that would idle the SDMA engines).
 - diag(w_j) stationaries for the PE path are built on-chip from a
   32 KB host-sent identity (tensor_scalar per-partition multiply) on
   DVE (block 1, needed first) and GpSimd (rest), replacing 0.64 MB of
   host-built diag slabs in the load stream.
 - Compute is spread over four lanes: PE runs blocks 1,3,5,7 + half of
   6 as diag(w_j) matmuls accumulating 4 taps into [128,2048] PSUM
   tiles (j-outer so a stationary serves 4 consecutive matmuls); DVE
   runs blocks 0,2,4 + half of 6 elementwise (shift-rebased products,
   pair-add tree); GpSimd computes the odd-shift products for blocks
   2,4,6h0 (odd fp16 offsets only reach DVE 2x mode, so they are cheap
   to offload) plus the store gates; ACT does only SiLU, in 2048-col
   chunks to amortize its 352-cycle per-instruction overhead.

Precision: x and the output are host-cast fp16 (halves HBM traffic both
ways); products and adds stay fp16 (PE accumulates fp32 in PSUM); SiLU
computes fp32-internally on ACT. End-to-end relative error ~5e-4.
"""

import sys
import types

import numpy as np

import concourse.bass as bass
import concourse.bacc as bacc
import concourse.mybir as mybir
from concourse.tile import TileContext
from concourse.bass_utils import run_bass_kernel_spmd


def _ensure_ntff_hook():
    """bass_utils imports antenv.axon_hooks when BASS_TRACE is set; that
    module is absent on this image. Install a shim so tracing works when
    possible and degrades gracefully (instead of crashing) when not."""
    try:
        import antenv.axon_hooks  # noqa: F401

        return
    except ImportError:
        pass
    try:
        import antenv

        hook = None
        try:
            if "/root/.axon_site" not in sys.path:
                sys.path.insert(0, "/root/.axon_site")
            from trn_agent_boot.trn_boot import _ntff_profile_via_ctypes

            hook = _ntff_profile_via_ctypes("/opt/axon/libaxon_pjrt.so")
        except Exception:
            hook = None
        mod = types.ModuleType("antenv.axon_hooks")
        mod._hook = hook
        mod.get_axon_ntff_profile_hook = lambda: mod._hook
        mod.set_axon_ntff_profile_hook = lambda h: setattr(mod, "_hook", h)
        sys.modules["antenv.axon_hooks"] = mod
        antenv.axon_hooks = mod
    except Exception:
        pass


_ensure_ntff_hook()

B, L, D = 4, 4096, 2048
K = 4
PAD = K - 1
N_CORES = 8
DH = D // 2            # channels per core
NBLK = DH // 128       # 128-partition channel blocks per core
ROWW = 4128            # DRAM row stride (fp16 elems): 64B-aligned rows
HALF = L // 2

MID_DT = mybir.dt.float16
PE_FULL = (1, 3, 5, 7)   # full blocks on the TensorEngine
PE_HALF_BLK = 6          # block 6: h0 on DVE, h1 on the PE
GPS_ODD = (2, 4)         # DVE blocks whose odd-shift products go to GpSimd
DIAG_BLKS = (1, 3, 5, 7, 6)

_cache = {}
_IDENT = np.eye(128, dtype=np.float16)


def _build_bass():
    nc = bacc.Bacc()
    xt = nc.dram_tensor("xt", [DH, ROWW], MID_DT, kind="ExternalInput")
    wt = nc.dram_tensor("wt", [128, NBLK * K], mybir.dt.float32, kind="ExternalInput")
    idt = nc.dram_tensor("idt", [128, 128], MID_DT, kind="ExternalInput")
    ot = nc.dram_tensor("ot", [DH, L], MID_DT, kind="ExternalOutput")
    f32 = mybir.dt.float32
    silu = mybir.ActivationFunctionType.Silu

    with TileContext(nc) as tc:
        with tc.tile_pool(name="pool", bufs=2) as pool, \
             tc.tile_pool(name="psum", bufs=2, space="PSUM") as psum_pool:
            # w + identity lead the sync ring so diag building can start
            # as soon as the first x block lands.
            w = pool.tile([128, NBLK * K], f32, tag="w", bufs=1)
            nc.sync.dma_start(out=w[:], in_=wt[:, :])
            ident = pool.tile([128, 128], MID_DT, tag="id", bufs=1)
            nc.sync.dma_start(out=ident[:], in_=idt[:, :])
            # Warmup: a tiny Silu forces the silu activation-table set to
            # load during the initial DMA wait; it is the only table load
            # in the whole kernel.
            warm = pool.tile([128, 2], MID_DT, tag="warm", bufs=1)
            nc.vector.memset(warm[:], 0.0)
            nc.scalar.activation(warm[:], warm[:], silu)

            # All x loads up front, back to back on the sync ring. The
            # first two blocks load in pieces so compute starts sooner;
            # x7 loads before x6 so the PE is fed in processing order.
            LOAD_ORDER = [0, 1, 2, 3, 4, 5, 7, 6]
            N_PIECES = {0: 2, 1: 2}
            xts = {}
            for blk in LOAD_ORDER:
                x = pool.tile([128, L + PAD + 1], MID_DT, tag="x", bufs=NBLK)
                n_p = N_PIECES.get(blk, 1)
                step = L // n_p
                cuts = [0] + [p * step + PAD for p in range(1, n_p)] + [L + PAD]
                for t0, t1 in zip(cuts[:-1], cuts[1:]):
                    nc.sync.dma_start(
                        out=x[:, t0:t1],
                        in_=xt[blk * 128 : (blk + 1) * 128, t0:t1],
                    )
                xts[blk] = x
            x_last = xts[LOAD_ORDER[-1]]

            # diag(w_j) stationaries, built on-chip: partition p of
            # diag[blk][:, j*128+p] = w[blk*128+p, j]. Block 1's slab is
            # needed first (PE starts on it) and is built on DVE; the
            # rest on GpSimd, which is otherwise idle early.
            diag = {}
            for blk in DIAG_BLKS:
                diag[blk] = pool.tile(
                    [128, K * 128], MID_DT, tag="wd", bufs=len(DIAG_BLKS),
                    name=f"wd{blk}",
                )
            for j in range(K):
                nc.vector.tensor_scalar_mul(
                    diag[1][:, j * 128 : (j + 1) * 128], ident[:],
                    w[:, 1 * K + j : 1 * K + j + 1],
                )
            for blk in (3, 5, 7, 6):
                for j in range(K):
                    nc.gpsimd.tensor_scalar_mul(
                        diag[blk][:, j * 128 : (j + 1) * 128], ident[:],
                        w[:, blk * K + j : blk * K + j + 1],
                    )

            def pe_half(blk, x, o, h0):
                # TensorEngine path for [h0, h0+HALF): one [128, 2048]
                # PSUM tile (4 banks), j-outer so one diag stationary
                # serves 4 consecutive 512-col matmuls; accumulation
                # groups interleave across the 4 regions (per-address
                # PSUM accumulate makes this valid). SiLU drains the
                # whole tile in one 2048-col ACT instruction.
                ps = psum_pool.tile([128, HALF], f32, tag="ps", bufs=2)
                for j in range(K):
                    lw = diag[blk][:, j * 128 : (j + 1) * 128]
                    for c in range(HALF // 512):
                        nc.tensor.matmul(
                            ps[:, c * 512 : (c + 1) * 512],
                            lw,
                            x[:, h0 + c * 512 + j : h0 + c * 512 + j + 512],
                            start=(j == 0),
                            stop=(j == K - 1),
                            skip_group_check=True,
                        )
                nc.scalar.activation(o[:, h0 : h0 + HALF], ps[:, :], silu)

            def dve_half(blk, x, o, h0, gps_odd):
                # Elementwise path for [h0, h0+HALF): qe holds the
                # even-shift products [q0 | q2], qo the odd [q1 | q3]
                # (on GpSimd when gps_odd - odd fp16 offsets only reach
                # DVE 2x mode anyway), pair-add + final add on DVE, one
                # 2048-col SiLU on ACT. Shift-rebased:
                # q_j[:, t] = w_j * x[:, h0 + t + j].
                wj = lambda j: w[:, blk * K + j : blk * K + j + 1]
                qe = pool.tile([128, 2, HALF], MID_DT, tag="qe", bufs=3)
                qo = pool.tile([128, 2, HALF], MID_DT, tag="qo", bufs=3)
                veng = nc.gpsimd if gps_odd else nc.vector
                veng.tensor_scalar_mul(qo[:, 0, :], x[:, h0 + 1 : h0 + 1 + HALF], wj(1))
                nc.vector.tensor_scalar_mul(qe[:, 0, :], x[:, h0 : h0 + HALF], wj(0))
                veng.tensor_scalar_mul(qo[:, 1, :], x[:, h0 + 3 : h0 + 3 + HALF], wj(3))
                nc.vector.tensor_scalar_mul(qe[:, 1, :], x[:, h0 + 2 : h0 + 2 + HALF], wj(2))
                nc.vector.tensor_add(qe[:, :, :], qe[:, :, :], qo[:, :, :])
                nc.vector.tensor_add(qe[:, 0, :], qe[:, 0, :], qe[:, 1, :])
                nc.scalar.activation(o[:, h0 : h0 + HALF], qe[:, 0, :], silu)

            # Per-block compute. Emission order = per-engine queue order:
            # block 7 before 6 so the ACT SiLU queue tail matches
            # readiness (x6 is the last load).
            os_ = {}
            for blk in [0, 1, 2, 3, 4, 5, 7, 6]:
                x = xts[blk]
                o = pool.tile([128, L], MID_DT, tag="o", bufs=NBLK)
                os_[blk] = o
                if blk in PE_FULL:
                    pe_half(blk, x, o, 0)
                    pe_half(blk, x, o, HALF)
                elif blk == PE_HALF_BLK:
                    dve_half(blk, x, o, 0, True)
                    pe_half(blk, x, o, HALF)
                else:
                    gps = blk in GPS_ODD
                    dve_half(blk, x, o, 0, gps)
                    dve_half(blk, x, o, HALF, gps)

            # Store gates + stores. Each gate is a 1-col GpSimd bypass
            # rewrite of the block's last output column whose scalar
            # operand is an fp32 token derived from the last-loaded x
            # tile: the store then has a data dependency on the final
            # load, so store descriptor-gen (and store HBM traffic)
            # cannot start until the load stream has drained - but
            # starts immediately after, unlike a DVE gate which would
            # sit behind the whole DVE compute queue.
            # Gate order approximates block completion order.
            tok = pool.tile([128, 1], f32, tag="tok", bufs=1)
            nc.gpsimd.tensor_scalar_mul(tok[:], x_last[:, 0:1], 0.0)
            for blk in [0, 1, 2, 3, 5, 4, 7, 6]:
                o = os_[blk]
                e = L - 1
                nc.gpsimd.tensor_scalar(
                    o[:, e : e + 1], o[:, e : e + 1], tok[:, 0:1], None,
                    mybir.AluOpType.bypass,
                )
                nc.sync.dma_start(
                    out=ot[blk * 128 : (blk + 1) * 128, :], in_=o[:, :]
                )
    nc.compile()
    return nc


def _shard_inputs(x, w):
    in_maps = []
    for core in range(N_CORES):
        b, half = divmod(core, 2)
        d0 = half * DH
        xt = np.zeros((DH, ROWW), dtype=np.float16)
        xt[:, PAD : PAD + L] = x[b, :, d0 : d0 + DH].T.astype(np.float16)
        # w rows for this shard, rearranged so partition p holds the K
        # weights of channel blk*128 + p at free cols [blk*K, blk*K + K)
        w_sh = w[d0 : d0 + DH].reshape(NBLK, 128, K)
        wt = (
            w_sh.transpose(1, 0, 2).reshape(128, NBLK * K).astype(np.float32)
        )
        in_maps.append(
            {
                "xt": np.ascontiguousarray(xt),
                "wt": np.ascontiguousarray(wt),
                "idt": _IDENT,
            }
        )
    return in_maps


def kernel(x, w):
    x = np.asarray(x, dtype=np.float32)
    w = np.asarray(w, dtype=np.float32)
    assert x.shape == (B, L, D) and w.shape == (D, K)

    if "nc" not in _cache:
        _cache["nc"] = _build_bass()
    nc = _cache["nc"]

    in_maps = _shard_inputs(x, w)
    res = None
    for attempt in range(3):
        try:
            res = run_bass_kernel_spmd(nc, in_maps, core_ids=list(range(N_CORES)))
            break
        except Exception:
            if attempt == 2:
                raise
    _cache["last_results"] = res

    out = np.empty((B, L, D), dtype=np.float32)
    for core in range(N_CORES):
        b, half = divmod(core, 2)
        d0 = half * DH
        out[b, :, d0 : d0 + DH] = res.results[core]["ot"].T.astype(np.float32)
    return out


# revision 7
# speedup vs baseline: 4.7794x; 4.7794x over previous
"""Depthwise causal Conv1d (k=4) + SiLU on 8 Trainium2 NeuronCores.

Problem: x [4, 4096, 2048] f32, w [2048, 4] f32,
out[b, t, d] = silu(sum_j w[d, j] * x[b, t - 3 + j, d])   (zero-padded left).

Sharding: 8 cores = 4 batches x 2 channel-halves. Depthwise conv is
independent per channel, so channel sharding needs no halo exchange.

Layout: each core receives its shard host-transposed to [channels, time]
(channels on SBUF partitions). The per-channel weight w[d, j] is a
per-partition scalar and the causal time shifts are free-dim AP offsets
into one loaded tile.

The kernel is HBM-bandwidth-bound: ~16.4 MB of fp16 I/O per core, and the
16 SDMA engines move ~1 MB each at their ~27 GiB/s line rate, so the floor
is startup (~7.5 us) + ~43 us of streaming + store-receipt/fence tail.
Schedule:
 - All loads queue back to back at the head of the sync HWDGE ring
   (pure reads, no stalls); per-block stores queue on the SAME ring
   afterwards, each gated by a tiny GpSimd bypass op that reads the
   LAST x tile, so store descriptor-gen starts right as the load stream
   drains (not earlier - that would steal load bandwidth and starve
   compute; not later - that would idle the SDMA engines).
 - diag(w_j) stationaries for the PE path are built on-chip from a
   32 KB host-sent identity (tensor_scalar per-partition multiply) on
   DVE (block 1, needed first) and GpSimd (rest), replacing 0.64 MB of
   host-built diag slabs in the load stream.
 - Compute is spread over four lanes: PE runs blocks 1,3,5,7 + half of
   6 as diag(w_j) matmuls accumulating 4 taps into [128,2048] PSUM
   tiles (j-outer so a stationary serves 4 consecutive matmuls); DVE
   runs blocks 0,2,4 + half of 6 elementwise (shift-rebased products,
   pair-add tree); GpSimd computes the odd-shift products for blocks
   2,4,6h0 (odd fp16 offsets only reach DVE 2x mode, so they are cheap
   to offload) plus the store gates; ACT does only SiLU, in 2048-col
   chunks to amortize its 352-cycle per-instruction overhead.

Precision: x and the output are host-cast fp16 (halves HBM traffic both
ways); products and adds stay fp16 (PE accumulates fp32 in PSUM); SiLU
computes fp32-internally on ACT. End-to-end relative error ~5e-4.
"""

import sys
import types

import numpy as np

import concourse.bass as bass
import concourse.bacc as bacc
import concourse.mybir as mybir
from concourse.tile import TileContext
from concourse.bass_utils import run_bass_kernel_spmd


def _ensure_ntff_hook():
    """bass_utils imports antenv.axon_hooks when BASS_TRACE is set; that
    module is absent on this image. Install a shim so tracing works when
    possible and degrades gracefully (instead of crashing) when not."""
    try:
        import antenv.axon_hooks  # noqa: F401

        return
    except ImportError:
        pass
    try:
        import antenv

        hook = None
        try:
            if "/root/.axon_site" not in sys.path:
                sys.path.insert(0, "/root/.axon_site")
            from trn_agent_boot.trn_boot import _ntff_profile_via_ctypes

            hook = _ntff_profile_via_ctypes("/opt/axon/libaxon_pjrt.so")
        except Exception:
            hook = None
        mod = types.ModuleType("antenv.axon_hooks")
        mod._hook = hook
        mod.get_axon_ntff_profile_hook = lambda: mod._hook
        mod.set_axon_ntff_profile_hook = lambda h: setattr(mod, "_hook", h)
        sys.modules["antenv.axon_hooks"] = mod
        antenv.axon_hooks = mod
    except Exception:
        pass


_ensure_ntff_hook()

B, L, D = 4, 4096, 2048
K = 4
PAD = K - 1
N_CORES = 8
DH = D // 2            # channels per core
NBLK = DH // 128       # 128-partition channel blocks per core
ROWW = 4128            # DRAM row stride (fp16 elems): 64B-aligned rows
HALF = L // 2

MID_DT = mybir.dt.float16
PE_FULL = (1, 3, 5, 7)   # full blocks on the TensorEngine
PE_HALF_BLK = 6          # block 6: h0 on DVE, h1 on the PE
GPS_ODD = (2, 4)         # DVE blocks whose odd-shift products go to GpSimd
DIAG_BLKS = (1, 3, 5, 7, 6)

_cache = {}
_IDENT = np.eye(128, dtype=np.float16)


def _build_bass():
    nc = bacc.Bacc()
    xt = nc.dram_tensor("xt", [DH, ROWW], MID_DT, kind="ExternalInput")
    wt = nc.dram_tensor("wt", [128, NBLK * K], mybir.dt.float32, kind="ExternalInput")
    idt = nc.dram_tensor("idt", [128, 128], MID_DT, kind="ExternalInput")
    ot = nc.dram_tensor("ot", [DH, L], MID_DT, kind="ExternalOutput")
    f32 = mybir.dt.float32
    silu = mybir.ActivationFunctionType.Silu

    with TileContext(nc) as tc:
        with tc.tile_pool(name="pool", bufs=2) as pool, \
             tc.tile_pool(name="psum", bufs=2, space="PSUM") as psum_pool:
            # w + identity lead the sync ring so diag building can start
            # as soon as the first x block lands.
            w = pool.tile([128, NBLK * K], f32, tag="w", bufs=1)
            nc.sync.dma_start(out=w[:], in_=wt[:, :])
            ident = pool.tile([128, 128], MID_DT, tag="id", bufs=1)
            nc.sync.dma_start(out=ident[:], in_=idt[:, :])
            # Warmup: a tiny Silu forces the silu activation-table set to
            # load during the initial DMA wait; it is the only table load
            # in the whole kernel.
            warm = pool.tile([128, 2], MID_DT, tag="warm", bufs=1)
            nc.vector.memset(warm[:], 0.0)
            nc.scalar.activation(warm[:], warm[:], silu)

            # All x loads up front, back to back on the sync ring. The
            # first two blocks load in pieces so compute starts sooner;
            # x7 loads before x6 so the PE is fed in processing order.
            LOAD_ORDER = [0, 1, 2, 3, 4, 5, 7, 6]
            N_PIECES = {0: 2, 1: 2}
            xts = {}
            for blk in LOAD_ORDER:
                x = pool.tile([128, L + PAD + 1], MID_DT, tag="x", bufs=NBLK)
                n_p = N_PIECES.get(blk, 1)
                step = L // n_p
                cuts = [0] + [p * step + PAD for p in range(1, n_p)] + [L + PAD]
                for t0, t1 in zip(cuts[:-1], cuts[1:]):
                    nc.sync.dma_start(
                        out=x[:, t0:t1],
                        in_=xt[blk * 128 : (blk + 1) * 128, t0:t1],
                    )
                xts[blk] = x
            x_last = xts[LOAD_ORDER[-1]]

            # diag(w_j) stationaries, built on-chip: partition p of
            # diag[blk][:, j*128+p] = w[blk*128+p, j]. Built on DVE
            # (GpSimd bulk elementwise is ~10x slower AND collides with
            # DVE on the shared SBUF port), interleaved into the DVE
            # queue so each slab lands just before the PE needs it.
            diag = {}
            for blk in DIAG_BLKS:
                diag[blk] = pool.tile(
                    [128, K * 128], MID_DT, tag="wd", bufs=len(DIAG_BLKS),
                    name=f"wd{blk}",
                )

            def build_diag(blk):
                for j in range(K):
                    nc.vector.tensor_scalar_mul(
                        diag[blk][:, j * 128 : (j + 1) * 128], ident[:],
                        w[:, blk * K + j : blk * K + j + 1],
                    )

            build_diag(1)

            def pe_half(blk, x, o, h0):
                # TensorEngine path for [h0, h0+HALF): one [128, 2048]
                # PSUM tile (4 banks), j-outer so one diag stationary
                # serves 4 consecutive 512-col matmuls; accumulation
                # groups interleave across the 4 regions (per-address
                # PSUM accumulate makes this valid). SiLU drains the
                # whole tile in one 2048-col ACT instruction.
                ps = psum_pool.tile([128, HALF], f32, tag="ps", bufs=2)
                for j in range(K):
                    lw = diag[blk][:, j * 128 : (j + 1) * 128]
                    for c in range(HALF // 512):
                        nc.tensor.matmul(
                            ps[:, c * 512 : (c + 1) * 512],
                            lw,
                            x[:, h0 + c * 512 + j : h0 + c * 512 + j + 512],
                            start=(j == 0),
                            stop=(j == K - 1),
                            skip_group_check=True,
                        )
                nc.scalar.activation(o[:, h0 : h0 + HALF], ps[:, :], silu)

            def dve_half(blk, x, o, h0, n_act=0):
                # Elementwise path for [h0, h0+HALF): qe holds the
                # even-shift products [q0 | q2], qo the odd [q1 | q3]
                # (n_act of the odd ones on ACT to relieve DVE), pair-add
                # + final add on DVE, one 2048-col SiLU on ACT.
                # Shift-rebased: q_j[:, t] = w_j * x[:, h0 + t + j].
                wj = lambda j: w[:, blk * K + j : blk * K + j + 1]
                qe = pool.tile([128, 2, HALF], MID_DT, tag="qe", bufs=3)
                qo = pool.tile([128, 2, HALF], MID_DT, tag="qo", bufs=3)
                if n_act >= 1:
                    nc.scalar.mul(qo[:, 0, :], x[:, h0 + 1 : h0 + 1 + HALF], wj(1))
                else:
                    nc.vector.tensor_scalar_mul(
                        qo[:, 0, :], x[:, h0 + 1 : h0 + 1 + HALF], wj(1)
                    )
                nc.vector.tensor_scalar_mul(qe[:, 0, :], x[:, h0 : h0 + HALF], wj(0))
                if n_act >= 2:
                    nc.scalar.mul(qo[:, 1, :], x[:, h0 + 3 : h0 + 3 + HALF], wj(3))
                else:
                    nc.vector.tensor_scalar_mul(
                        qo[:, 1, :], x[:, h0 + 3 : h0 + 3 + HALF], wj(3)
                    )
                nc.vector.tensor_scalar_mul(qe[:, 1, :], x[:, h0 + 2 : h0 + 2 + HALF], wj(2))
                nc.vector.tensor_add(qe[:, :, :], qe[:, :, :], qo[:, :, :])
                nc.vector.tensor_add(qe[:, 0, :], qe[:, 0, :], qe[:, 1, :])
                nc.scalar.activation(o[:, h0 : h0 + HALF], qe[:, 0, :], silu)

            # Per-block compute. Emission order = per-engine queue order:
            # block 7 before 6 so the ACT SiLU queue tail matches
            # readiness (x6 is the last load). Remaining diag slabs are
            # slotted into the DVE queue between early halves, each well
            # before the PE reaches that block. ACT takes the q1 product
            # of the earliest DVE halves (it is idle before SiLUs pile
            # up; DVE is the tighter budget).
            ACT_ODD = {(0, 0): 1, (0, 1): 1, (2, 0): 1}
            DIAG_AFTER = {(0, 0): 3, (0, 1): 5, (2, 0): 7, (2, 1): 6}
            os_ = {}
            for blk in [0, 1, 2, 3, 4, 5, 7, 6]:
                x = xts[blk]
                o = pool.tile([128, L], MID_DT, tag="o", bufs=NBLK)
                os_[blk] = o
                if blk in PE_FULL:
                    pe_half(blk, x, o, 0)
                    pe_half(blk, x, o, HALF)
                elif blk == PE_HALF_BLK:
                    dve_half(blk, x, o, 0)
                    pe_half(blk, x, o, HALF)
                else:
                    for h in (0, 1):
                        dve_half(blk, x, o, h * HALF, ACT_ODD.get((blk, h), 0))
                        nxt = DIAG_AFTER.get((blk, h))
                        if nxt is not None:
                            build_diag(nxt)

            # Store gates + stores. Each gate is a 1-col GpSimd bypass
            # rewrite of the block's last output column whose scalar
            # operand is an fp32 token derived from the last-loaded x
            # tile: the store then has a data dependency on the final
            # load, so store descriptor-gen (and store HBM traffic)
            # cannot start until the load stream has drained - but
            # starts immediately after, unlike a DVE gate which would
            # sit behind the whole DVE compute queue.
            # Gate order approximates block completion order.
            tok = pool.tile([128, 1], f32, tag="tok", bufs=1)
            nc.gpsimd.tensor_scalar_mul(tok[:], x_last[:, 0:1], 0.0)
            for blk in [0, 1, 2, 3, 5, 4, 7, 6]:
                o = os_[blk]
                e = L - 1
                nc.gpsimd.tensor_scalar(
                    o[:, e : e + 1], o[:, e : e + 1], tok[:, 0:1], None,
                    mybir.AluOpType.bypass,
                )
                nc.sync.dma_start(
                    out=ot[blk * 128 : (blk + 1) * 128, :], in_=o[:, :]
                )
    nc.compile()
    return nc


def _shard_inputs(x, w):
    in_maps = []
    for core in range(N_CORES):
        b, half = divmod(core, 2)
        d0 = half * DH
        xt = np.zeros((DH, ROWW), dtype=np.float16)
        xt[:, PAD : PAD + L] = x[b, :, d0 : d0 + DH].T.astype(np.float16)
        # w rows for this shard, rearranged so partition p holds the K
        # weights of channel blk*128 + p at free cols [blk*K, blk*K + K)
        w_sh = w[d0 : d0 + DH].reshape(NBLK, 128, K)
        wt = (
            w_sh.transpose(1, 0, 2).reshape(128, NBLK * K).astype(np.float32)
        )
        in_maps.append(
            {
                "xt": np.ascontiguousarray(xt),
                "wt": np.ascontiguousarray(wt),
                "idt": _IDENT,
            }
        )
    return in_maps


def kernel(x, w):
    x = np.asarray(x, dtype=np.float32)
    w = np.asarray(w, dtype=np.float32)
    assert x.shape == (B, L, D) and w.shape == (D, K)

    if "nc" not in _cache:
        _cache["nc"] = _build_bass()
    nc = _cache["nc"]

    in_maps = _shard_inputs(x, w)
    res = None
    for attempt in range(3):
        try:
            res = run_bass_kernel_spmd(nc, in_maps, core_ids=list(range(N_CORES)))
            break
        except Exception:
            if attempt == 2:
                raise
    _cache["last_results"] = res

    out = np.empty((B, L, D), dtype=np.float32)
    for core in range(N_CORES):
        b, half = divmod(core, 2)
        d0 = half * DH
        out[b, :, d0 : d0 + DH] = res.results[core]["ot"].T.astype(np.float32)
    return out
